# revision 26
# baseline (speedup 1.0000x reference)
"""Trainium2 Bass kernel for CalibratedProjectiveLinear (QINS log-quantized linear).

y = (x @ W^T + bias) * scale, with W reconstructed elementwise from a
log-scale uint8 encoding: W[o,i] = sign[o,i] * exp(log_min + (255-stored[o,i])/254
* (log_max-log_min)).

Sharding: column-parallel over out_features across 8 cores. x is replicated
(passed transposed so the contraction dim lands on SBUF partitions); stored/
sign are passed per-shard transposed AND group-blocked so every weight DMA is
a single fully-linear read (host-side layout transform only — byte count is
unchanged). Each core computes y_shard^T = [O_SH, B]; the host concatenates
and transposes back.

Device pipeline per core, per contraction super-chunk (CHUNK x 128 rows):
  linear DMA int32 stored/sign -> ACT: exp(c1*stored + c0) int32->fp32
  -> DVE: multiply by sign (mixed int32 operand) -> PE: float32r matmuls
  accumulating into PSUM over the 32 contraction chunks; bias is folded in
  as a final rank-1 (ones x bias) accumulation; per-channel scale applied by
  ACT Copy (per-partition vector in the [o, b] output orientation).
"""

import numpy as np

import concourse.bass as bass
import concourse.mybir as mybir
from concourse import tile
from concourse.bass_utils import run_bass_kernel_spmd

B, IN, OUT = 512, 4096, 11008
N_CORES = 8
O_SH = OUT // N_CORES            # 1376 out-features per core
K_TILES = IN // 128              # 32 contraction chunks
O_TILE_WIDTHS = [128] * (O_SH // 128) + ([O_SH % 128] if O_SH % 128 else [])
N_OT = len(O_TILE_WIDTHS)        # 11 (10x128 + 96)
O_GROUPS = [list(range(0, 4)), list(range(4, 8)), list(range(8, N_OT))]
CHUNK = 2                        # contraction chunks per weight DMA
FP32 = mybir.dt.float32
FP32R = mybir.dt.float32r
INT32 = mybir.dt.int32

_COMPILED = {}


def _group_geometry():
    o_offs = np.cumsum([0] + O_TILE_WIDTHS).tolist()
    geo = []
    blk_off = 0
    for group in O_GROUPS:
        g0 = o_offs[group[0]]
        gw = o_offs[group[-1] + 1] - g0
        geo.append((group, g0, gw, blk_off))
        blk_off += IN * gw
    return o_offs, geo


def _split_multi_waits(nc: bass.Bass) -> int:
    """The walrus build in this container accepts at most ONE sync wait per
    instruction; Tile freely emits several. Split extras into single-wait
    NoOps on the same engine, inserted just before the instruction
    (semantically identical: all waits must pass before it executes)."""
    n_split = 0
    for blk in nc.main_func.blocks:
        new_insts = []
        for inst in blk.instructions:
            si = inst.sync_info
            if si is not None and len(si.on_wait) > 1:
                waits = list(si.on_wait)
                for w in waits[:-1]:
                    nop = mybir.InstNoOp(
                        name=nc.get_next_instruction_name(), ins=[], outs=[])
                    nop.engine = inst.engine
                    nop.sync_info = mybir.SyncInfo(on_wait=[w], on_update=[])
                    nc.register_instruction(nop)
                    new_insts.append(nop)
                    n_split += 1
                inst.sync_info = mybir.SyncInfo(
                    on_wait=[waits[-1]], on_update=list(si.on_update))
            new_insts.append(inst)
        blk.instructions = new_insts
    return n_split


def _build(c0: float, c1: float, repeat: int = 1, variant: str = "full",
           stage_bufs: int = 3) -> bass.Bass:
    mmdt = mybir.dt.bfloat16 if variant == "bf16" else FP32R
    nc = bass.Bass()
    storedB = nc.dram_tensor("storedB", [IN * O_SH], INT32, kind="ExternalInput")
    signB = nc.dram_tensor("signB", [IN * O_SH], INT32, kind="ExternalInput")
    xT = nc.dram_tensor("xT", [IN, B], FP32, kind="ExternalInput")
    scale_m = nc.dram_tensor("scale_m", [128, N_OT], FP32, kind="ExternalInput")
    bias_r = nc.dram_tensor("bias_r", [1, O_SH], FP32, kind="ExternalInput")
    out = nc.dram_tensor("out", [O_SH, B], FP32, kind="ExternalOutput")

    with tile.TileContext(nc) as tc:
        with (
            tc.tile_pool(name="consts", bufs=1) as consts,
            tc.tile_pool(name="stage", bufs=stage_bufs) as stage,
            tc.tile_pool(name="resp", bufs=3) as resp,
            tc.tile_pool(name="xstage", bufs=2) as xstage,
            tc.tile_pool(name="psum", bufs=1, space="PSUM") as psum,
        ):
            ones_f = consts.tile([1, B], FP32)
            nc.vector.memset(ones_f[:], 1.0)
            ones = consts.tile([1, B], mmdt)
            nc.vector.tensor_copy(ones[:], ones_f[:])
            c0_t = consts.tile([128, 1], FP32)
            nc.vector.memset(c0_t[:], c0)
            scale_t = consts.tile([128, N_OT], FP32)
            nc.sync.dma_start(scale_t[:], scale_m[:])
            bias_f = consts.tile([1, O_SH], FP32)
            nc.sync.dma_start(bias_f[:], bias_r[:])
            bias_t = consts.tile([1, O_SH], mmdt)
            nc.vector.tensor_copy(bias_t[:], bias_f[:])

            x_tiles = {}

            def get_x(i):
                # Lazy: emitted at first use so the weight-stream DMAs are
                # not queued behind the full 8.4 MB x preload at kernel
                # start. For repeat>1 all tiles are pre-emitted outside the
                # loop (below), so the loop body slope measures steady state.
                if i not in x_tiles:
                    xf = xstage.tile([128, B], FP32, tag="xf", name=f"xf_{i}")
                    nc.sync.dma_start(xf[:], xT[i * 128:(i + 1) * 128, :])
                    xt = consts.tile([128, B], mmdt, tag=f"x{i}", name=f"x_{i}")
                    nc.vector.tensor_copy(xt[:], xf[:])
                    x_tiles[i] = xt
                return x_tiles[i]

            if repeat != 1:
                for i in range(K_TILES):
                    get_x(i)

            o_offs, geo = _group_geometry()

            def body():
                emit_groups(nc, o_offs, geo, storedB, signB, out,
                            get_x, ones, bias_t, scale_t, c0_t, c1,
                            stage, resp, psum, variant)

            if repeat == 1:
                body()
            else:
                with tc.For_i(0, repeat, 1):
                    body()

    _split_multi_waits(nc)
    nc.finalize()
    return nc


def emit_groups(nc, o_offs, geo, storedB, signB, out, get_x, ones, bias_t,
                scale_t, c0_t, c1, stage, resp, psum, variant="full"):
    wdt = mybir.dt.bfloat16 if variant == "bf16" else FP32R

    def make_tail(group, accs):
        # group output path: bias rank-1 accumulation, per-channel scale via
        # ACT Copy (PSUM -> SBUF), store. Emitted AFTER the next group's
        # pipeline has started so the in-order ACT queue never stalls on the
        # bias matmul.
        def tail():
            for t in group:
                tw = O_TILE_WIDTHS[t]
                oo = o_offs[t]
                nc.tensor.matmul(
                    accs[t][:],
                    bias_t[:, oo:oo + tw],
                    ones[:],
                    start=False, stop=True,
                )
                if variant == "nout":
                    continue
                res = resp.tile([128, B], FP32, tag="res", name=f"res_{t}")
                nc.vector.tensor_scalar(res[:tw, :], accs[t][:],
                                        scale_t[:tw, t:t + 1], None,
                                        mybir.AluOpType.mult)
                nc.sync.dma_start(out[oo:oo + tw, :], res[:tw, :])
        return tail

    pending_tail = None
    for group, g0, gw, blk in geo:
        if variant != "nope":
            accs = {t: psum.tile([O_TILE_WIDTHS[t], B], FP32,
                                 name=f"acc_{t}", tag=f"acc{t % 8}")
                    for t in group}
        for ib in range(K_TILES // CHUNK):
            # one fully-linear DMA covering CHUNK contraction chunks
            span = CHUNK * 128 * gw
            src_st = storedB[blk + ib * span: blk + (ib + 1) * span]
            src_sg = signB[blk + ib * span: blk + (ib + 1) * span]
            st = stage.tile([128, CHUNK, gw], INT32, tag="st")
            nc.sync.dma_start(st[:], src_st.rearrange("(a p b) -> p a b",
                                                      p=128, b=gw))
            sg = stage.tile([128, CHUNK, gw], INT32, tag="sg")
            nc.sync.dma_start(sg[:], src_sg.rearrange("(a p b) -> p a b",
                                                      p=128, b=gw))
            if variant == "dma":
                continue
            wmag = stage.tile([128, CHUNK, gw], FP32, tag="wmag")
            nc.scalar.activation(wmag[:], st[:], mybir.ActivationFunctionType.Exp,
                                 bias=c0_t[:], scale=c1)
            w = stage.tile([128, CHUNK, gw], wdt, tag="w")
            nc.vector.tensor_mul(w[:], wmag[:], sg[:])
            if variant == "nope":
                continue
            for j in range(CHUNK):
                i = ib * CHUNK + j
                for t in group:
                    tw = O_TILE_WIDTHS[t]
                    toff = o_offs[t] - g0
                    nc.tensor.matmul(
                        accs[t][:],
                        w[:, j, toff:toff + tw],
                        get_x(i)[:],
                        start=(i == 0), stop=False,
                    )
            if ib == 1 and pending_tail is not None:
                pending_tail()
                pending_tail = None
        if variant in ("nope", "dma"):
            continue
        pending_tail = make_tail(group, accs)
    if pending_tail is not None:
        pending_tail()


def _blocked(mT: np.ndarray) -> np.ndarray:
    """[IN, O_SH] -> flat group-blocked layout (each group's columns stored
    as a contiguous [IN, gw] block)."""
    _, geo = _group_geometry()
    parts = [np.ascontiguousarray(mT[:, g0:g0 + gw]).ravel()
             for _, g0, gw, _ in geo]
    return np.concatenate(parts)


def kernel(x, stored, sign, log_min, log_max, scale, bias):
    log_min = float(np.asarray(log_min))
    log_max = float(np.asarray(log_max))
    # exp(log_min + (255 - s)/254 * d) == exp(c0 + c1*s)
    d = log_max - log_min
    c1 = -d / 254.0
    c0 = log_min + 255.0 * d / 254.0

    key = (c0, c1)
    if key not in _COMPILED:
        _COMPILED[key] = _build(c0, c1)
    nc = _COMPILED[key]

    xT = np.ascontiguousarray(np.asarray(x, dtype=np.float32).T)
    stored = np.asarray(stored, dtype=np.int32)
    sign = np.asarray(sign, dtype=np.int32)
    scale = np.asarray(scale, dtype=np.float32)
    bias = np.asarray(bias, dtype=np.float32)

    in_maps = []
    for c in range(N_CORES):
        o0, o1 = c * O_SH, (c + 1) * O_SH
        scale_pad = np.ones(N_OT * 128, dtype=np.float32)
        scale_pad[:O_SH] = scale[o0:o1]
        in_maps.append({
            "storedB": _blocked(stored[o0:o1].T),
            "signB": _blocked(sign[o0:o1].T),
            "xT": xT,
            "scale_m": np.ascontiguousarray(scale_pad.reshape(N_OT, 128).T),
            "bias_r": np.ascontiguousarray(bias[o0:o1].reshape(1, O_SH)),
        })

    global _last_in_maps
    _last_in_maps = in_maps
    res = run_bass_kernel_spmd(nc, in_maps, list(range(N_CORES)))
    yT = np.concatenate([res.results[c]["out"] for c in range(N_CORES)], axis=0)
    return np.ascontiguousarray(yT.T)


# revision 28
# speedup vs baseline: 1.0316x; 1.0316x over previous
"""Trainium2 Bass kernel for CalibratedProjectiveLinear (QINS log-quantized linear).

y = (x @ W^T + bias) * scale, with W reconstructed elementwise from a
log-scale uint8 encoding: W[o,i] = sign[o,i] * exp(log_min + (255-stored[o,i])/254
* (log_max-log_min)).

Sharding: column-parallel over out_features across 8 cores. x is replicated
(passed transposed so the contraction dim lands on SBUF partitions); stored/
sign are passed per-shard transposed AND group-blocked so every weight DMA is
a single fully-linear read (host-side layout transform only — byte count is
unchanged). Each core computes y_shard^T = [O_SH, B]; the host concatenates
and transposes back.

Device pipeline per core, per contraction super-chunk (CHUNK x 128 rows):
  linear DMA int32 stored/sign -> ACT: exp(c1*stored + c0) int32->fp32
  -> DVE: multiply by sign (mixed int32 operand) -> PE: float32r matmuls
  accumulating into PSUM over the 32 contraction chunks; bias is folded in
  as a final rank-1 (ones x bias) accumulation; per-channel scale applied
  during the PSUM->SBUF evacuation (DVE tensor_scalar, per-partition vector
  in the [o, b] output orientation). All output stores are held in SBUF and
  issued at the end of the body so the weight-read stream is never
  interleaved with HBM writes (measured ~9 us/pass win).
"""

import numpy as np

import concourse.bass as bass
import concourse.mybir as mybir
from concourse import tile
from concourse.bass_utils import run_bass_kernel_spmd

B, IN, OUT = 512, 4096, 11008
N_CORES = 8
O_SH = OUT // N_CORES            # 1376 out-features per core
K_TILES = IN // 128              # 32 contraction chunks
O_TILE_WIDTHS = [128] * (O_SH // 128) + ([O_SH % 128] if O_SH % 128 else [])
N_OT = len(O_TILE_WIDTHS)        # 11 (10x128 + 96)
O_GROUPS = [list(range(0, 4)), list(range(4, 8)), list(range(8, N_OT))]
CHUNK = 2                        # contraction chunks per weight DMA
FP32 = mybir.dt.float32
FP32R = mybir.dt.float32r
INT32 = mybir.dt.int32

_COMPILED = {}


def _group_geometry():
    o_offs = np.cumsum([0] + O_TILE_WIDTHS).tolist()
    geo = []
    blk_off = 0
    for group in O_GROUPS:
        g0 = o_offs[group[0]]
        gw = o_offs[group[-1] + 1] - g0
        geo.append((group, g0, gw, blk_off))
        blk_off += IN * gw
    return o_offs, geo


def _split_multi_waits(nc: bass.Bass) -> int:
    """The walrus build in this container accepts at most ONE sync wait per
    instruction; Tile freely emits several. Split extras into single-wait
    NoOps on the same engine, inserted just before the instruction
    (semantically identical: all waits must pass before it executes)."""
    n_split = 0
    for blk in nc.main_func.blocks:
        new_insts = []
        for inst in blk.instructions:
            si = inst.sync_info
            if si is not None and len(si.on_wait) > 1:
                waits = list(si.on_wait)
                for w in waits[:-1]:
                    nop = mybir.InstNoOp(
                        name=nc.get_next_instruction_name(), ins=[], outs=[])
                    nop.engine = inst.engine
                    nop.sync_info = mybir.SyncInfo(on_wait=[w], on_update=[])
                    nc.register_instruction(nop)
                    new_insts.append(nop)
                    n_split += 1
                inst.sync_info = mybir.SyncInfo(
                    on_wait=[waits[-1]], on_update=list(si.on_update))
            new_insts.append(inst)
        blk.instructions = new_insts
    return n_split


def _build(c0: float, c1: float, repeat: int = 1, variant: str = "wend",
           stage_bufs: int = 3) -> bass.Bass:
    mmdt = mybir.dt.bfloat16 if variant == "bf16" else FP32R
    nc = bass.Bass()
    storedB = nc.dram_tensor("storedB", [IN * O_SH], INT32, kind="ExternalInput")
    signB = nc.dram_tensor("signB", [IN * O_SH], INT32, kind="ExternalInput")
    xT = nc.dram_tensor("xT", [IN, B], FP32, kind="ExternalInput")
    scale_m = nc.dram_tensor("scale_m", [128, N_OT], FP32, kind="ExternalInput")
    bias_r = nc.dram_tensor("bias_r", [1, O_SH], FP32, kind="ExternalInput")
    out = nc.dram_tensor("out", [O_SH, B], FP32, kind="ExternalOutput")

    with tile.TileContext(nc) as tc:
        with (
            tc.tile_pool(name="consts", bufs=1) as consts,
            tc.tile_pool(name="stage", bufs=stage_bufs) as stage,
            tc.tile_pool(name="resp", bufs=3) as resp,
            tc.tile_pool(name="xstage", bufs=2) as xstage,
            tc.tile_pool(name="psum", bufs=1, space="PSUM") as psum,
        ):
            ones_f = consts.tile([1, B], FP32)
            nc.vector.memset(ones_f[:], 1.0)
            ones = consts.tile([1, B], mmdt)
            nc.vector.tensor_copy(ones[:], ones_f[:])
            c0_t = consts.tile([128, 1], FP32)
            nc.vector.memset(c0_t[:], c0)
            scale_t = consts.tile([128, N_OT], FP32)
            nc.sync.dma_start(scale_t[:], scale_m[:])
            bias_f = consts.tile([1, O_SH], FP32)
            nc.sync.dma_start(bias_f[:], bias_r[:])
            bias_t = consts.tile([1, O_SH], mmdt)
            nc.vector.tensor_copy(bias_t[:], bias_f[:])

            x_tiles = {}

            def get_x(i):
                # Lazy: emitted at first use so the weight-stream DMAs are
                # not queued behind the full 8.4 MB x preload at kernel
                # start. For repeat>1 all tiles are pre-emitted outside the
                # loop (below), so the loop body slope measures steady state.
                if i not in x_tiles:
                    xf = xstage.tile([128, B], FP32, tag="xf", name=f"xf_{i}")
                    nc.sync.dma_start(xf[:], xT[i * 128:(i + 1) * 128, :])
                    xt = consts.tile([128, B], mmdt, tag=f"x{i}", name=f"x_{i}")
                    nc.vector.tensor_copy(xt[:], xf[:])
                    x_tiles[i] = xt
                return x_tiles[i]

            if repeat != 1:
                for i in range(K_TILES):
                    get_x(i)

            o_offs, geo = _group_geometry()

            def body():
                emit_groups(nc, o_offs, geo, storedB, signB, out,
                            get_x, ones, bias_t, scale_t, c0_t, c1,
                            stage, resp, psum, variant)

            if repeat == 1:
                body()
            else:
                with tc.For_i(0, repeat, 1):
                    body()

    _split_multi_waits(nc)
    nc.finalize()
    return nc


def emit_groups(nc, o_offs, geo, storedB, signB, out, get_x, ones, bias_t,
                scale_t, c0_t, c1, stage, resp, psum, variant="full"):
    wdt = mybir.dt.bfloat16 if variant == "bf16" else FP32R

    late_stores = []

    def make_tail(group, accs):
        # group output path: bias rank-1 accumulation, per-channel scale
        # applied during the PSUM -> SBUF evacuation (DVE tensor_scalar with
        # a per-partition vector), then store. Emitted AFTER the next
        # group's pipeline has started so in-order engine queues never stall
        # on the bias matmul. variant "wend": stores are held until the end
        # of the body so the weight-read stream is never interleaved with
        # HBM writes.
        def tail():
            for t in group:
                tw = O_TILE_WIDTHS[t]
                oo = o_offs[t]
                nc.tensor.matmul(
                    accs[t][:],
                    bias_t[:, oo:oo + tw],
                    ones[:],
                    start=False, stop=True,
                )
                if variant == "nout":
                    continue
                tag = f"res{t}" if variant == "wend" else "res"
                res = resp.tile([128, B], FP32, tag=tag, name=f"res_{t}")
                nc.vector.tensor_scalar(res[:tw, :], accs[t][:],
                                        scale_t[:tw, t:t + 1], None,
                                        mybir.AluOpType.mult)
                if variant == "wend":
                    late_stores.append((oo, tw, res))
                else:
                    nc.sync.dma_start(out[oo:oo + tw, :], res[:tw, :])
        return tail

    pending_tail = None
    for group, g0, gw, blk in geo:
        if variant != "nope":
            accs = {t: psum.tile([O_TILE_WIDTHS[t], B], FP32,
                                 name=f"acc_{t}", tag=f"acc{t % 8}")
                    for t in group}
        for ib in range(K_TILES // CHUNK):
            # one fully-linear DMA covering CHUNK contraction chunks
            span = CHUNK * 128 * gw
            src_st = storedB[blk + ib * span: blk + (ib + 1) * span]
            src_sg = signB[blk + ib * span: blk + (ib + 1) * span]
            st = stage.tile([128, CHUNK, gw], INT32, tag="st")
            nc.sync.dma_start(st[:], src_st.rearrange("(a p b) -> p a b",
                                                      p=128, b=gw))
            sg = stage.tile([128, CHUNK, gw], INT32, tag="sg")
            nc.sync.dma_start(sg[:], src_sg.rearrange("(a p b) -> p a b",
                                                      p=128, b=gw))
            if variant == "dma":
                continue
            wmag = stage.tile([128, CHUNK, gw], FP32, tag="wmag")
            nc.scalar.activation(wmag[:], st[:], mybir.ActivationFunctionType.Exp,
                                 bias=c0_t[:], scale=c1)
            w = stage.tile([128, CHUNK, gw], wdt, tag="w")
            nc.vector.tensor_mul(w[:], wmag[:], sg[:])
            if variant == "nope":
                continue
            for j in range(CHUNK):
                i = ib * CHUNK + j
                for t in group:
                    tw = O_TILE_WIDTHS[t]
                    toff = o_offs[t] - g0
                    nc.tensor.matmul(
                        accs[t][:],
                        w[:, j, toff:toff + tw],
                        get_x(i)[:],
                        start=(i == 0), stop=False,
                    )
            if ib == 1 and pending_tail is not None:
                pending_tail()
                pending_tail = None
        if variant in ("nope", "dma"):
            continue
        pending_tail = make_tail(group, accs)
    if pending_tail is not None:
        pending_tail()
    for oo, tw, res in late_stores:
        nc.sync.dma_start(out[oo:oo + tw, :], res[:tw, :])


def _blocked(mT: np.ndarray) -> np.ndarray:
    """[IN, O_SH] -> flat group-blocked layout (each group's columns stored
    as a contiguous [IN, gw] block)."""
    _, geo = _group_geometry()
    parts = [np.ascontiguousarray(mT[:, g0:g0 + gw]).ravel()
             for _, g0, gw, _ in geo]
    return np.concatenate(parts)


def kernel(x, stored, sign, log_min, log_max, scale, bias):
    log_min = float(np.asarray(log_min))
    log_max = float(np.asarray(log_max))
    # exp(log_min + (255 - s)/254 * d) == exp(c0 + c1*s)
    d = log_max - log_min
    c1 = -d / 254.0
    c0 = log_min + 255.0 * d / 254.0

    key = (c0, c1)
    if key not in _COMPILED:
        _COMPILED[key] = _build(c0, c1)
    nc = _COMPILED[key]

    xT = np.ascontiguousarray(np.asarray(x, dtype=np.float32).T)
    stored = np.asarray(stored, dtype=np.int32)
    sign = np.asarray(sign, dtype=np.int32)
    scale = np.asarray(scale, dtype=np.float32)
    bias = np.asarray(bias, dtype=np.float32)

    in_maps = []
    for c in range(N_CORES):
        o0, o1 = c * O_SH, (c + 1) * O_SH
        scale_pad = np.ones(N_OT * 128, dtype=np.float32)
        scale_pad[:O_SH] = scale[o0:o1]
        in_maps.append({
            "storedB": _blocked(stored[o0:o1].T),
            "signB": _blocked(sign[o0:o1].T),
            "xT": xT,
            "scale_m": np.ascontiguousarray(scale_pad.reshape(N_OT, 128).T),
            "bias_r": np.ascontiguousarray(bias[o0:o1].reshape(1, O_SH)),
        })

    global _last_in_maps
    _last_in_maps = in_maps
    res = run_bass_kernel_spmd(nc, in_maps, list(range(N_CORES)))
    yT = np.concatenate([res.results[c]["out"] for c in range(N_CORES)], axis=0)
    return np.ascontiguousarray(yT.T)
